# revision 14
# baseline (speedup 1.0000x reference)
"""Trainium2 Bass kernel for nn_CorrBlock: softmax(fmap1 @ fmap2.T / sqrt(D), axis=-1).

Sharding: fmap1 rows split across 8 cores (1024 rows each), fmap2 replicated.
Each core computes its [1024, 8192] slab of the output independently.

Device kernel (per core):
  - Inputs are pre-transposed on the host to [128, D/128, rows] so the
    contraction dim lands on SBUF partitions with no on-device transpose.
  - PE: matmuls accumulate the D=256 contraction in 2 chunks of 128 into PSUM.
  - ACT: Exp with fused 1/sqrt(D) scale reads PSUM, writes fp16 SBUF, and
    emits per-row partial sums via accum_out in the same pass.
  - DVE: reciprocal of the row sum, then per-row scalar multiply in fp16
    (2x/4x DVE mode).
  - DMA out the normalized [128, 8192] block as fp16; the host upcasts to
    fp32. fp16 quantization of softmax probs is ~5e-4 rel err, far inside
    tolerance, and halves the dominant output DMA traffic.

Schedule (v2):
  - ~2.8us of dummy matmuls on a memset scratch tile, issued before any
    DMA-dependent work, so the PE HAM clock-gate opens (1.2 -> 2.4 GHz)
    before the first real matmul instead of ~7us into the run.
  - Block 0's first column chunk is activated in 512-col pieces so the
    scalar engine starts exp'ing as soon as the first 512 f2 columns land,
    instead of waiting for a full 2048-col chunk.
  - The ACT engine is the critical resource (8.4M exps at 1 elem/cycle
    @1.2GHz = ~55us + per-instruction overhead); everything else (PE, DMA,
    DVE) is scheduled to hide under the gap-free ACTIVATE stream.
"""

import os
import sys

import numpy as np

if "/opt/trn_rl_repo" not in sys.path:
    sys.path.insert(0, "/opt/trn_rl_repo")

import concourse.bacc as bacc
import concourse.bass as bass
import concourse.mybir as mybir
import concourse.tile as tile
from concourse.bass_utils import run_bass_kernel_spmd

N, M, D = 8192, 8192, 256
N_CORES = 8
NB = N // N_CORES  # rows per core
DC = D // 128  # contraction chunks
QC = 2048  # columns handled per PSUM tile (4 banks)

MM_DT = os.environ.get("CORR_MM_DT", "float16")
N_WARM = int(os.environ.get("CORR_N_WARM", "20"))
SUMS_ON = os.environ.get("CORR_SUMS", "act")  # "act" (accum_out) | "dve"
N_SPLIT = int(os.environ.get("CORR_NSPLIT", "4"))  # blocks with split q0

# Populated by kernel() on every run (exec_time_ns only when tracing).
last_run_info: dict = {}


def _chunks(m):
    """Uniform 2048-wide column chunks (4 PSUM banks each)."""
    if m % 2048:
        return [m]
    return [2048] * (m // 2048)


def build_nc(nb=NB, m=M, dc=DC, qc=QC, mm_dt=None):
    """Build the per-core Bass program. Shapes in elements."""
    f32 = mybir.dt.float32
    f16 = mybir.dt.float16
    mm_dtype = getattr(mybir.dt, mm_dt or MM_DT)
    n_blocks = nb // 128
    chunks = _chunks(m)
    n_q = len(chunks)
    coff = [sum(chunks[:i]) for i in range(n_q + 1)]  # column offsets
    scale = 1.0 / (D**0.5)

    nc = bacc.Bacc("TRN2", target_bir_lowering=False, debug=False)

    f1t = nc.dram_tensor("f1t", [128, dc, nb], mm_dtype, kind="ExternalInput")
    f2t = nc.dram_tensor("f2t", [128, dc, m], mm_dtype, kind="ExternalInput")
    out = nc.dram_tensor("out", [nb, m], f16, kind="ExternalOutput")

    with tile.TileContext(nc) as tc:
        with (
            tc.tile_pool(name="weights", bufs=1) as wpool,
            tc.tile_pool(name="exps", bufs=n_blocks) as epool,
            tc.tile_pool(name="stats", bufs=n_blocks) as spool,
            tc.tile_pool(name="psum", bufs=2, space="PSUM") as ppool,
        ):
            # PE warm-up: the HAM clock gate only opens to 2.4 GHz after
            # ~3.4us of sustained PE activity.  Run dummy matmuls on a
            # memset scratch tile (no DMA dependency, so they start right
            # after the engine preamble) so the ramp overlaps the input
            # DMA stream and real matmuls run warm almost from the start.
            ps_warm = ppool.tile([128, chunks[0] // 512, 512], f32, tag="ps")
            if N_WARM:
                scratch = wpool.tile([128, 128], mm_dtype, tag="scratch")
                nc.gpsimd.memset(scratch[:], 0)
                for w in range(N_WARM):
                    nc.tensor.matmul(
                        ps_warm[:, 0, 0:128],
                        scratch[:],
                        scratch[:],
                        start=(w == 0),
                        stop=False,
                    )

            # Stage the input DMAs on TWO hardware queues (Sync + GpSimd,
            # ~260 GB/s each) in consumption order.  f1 rides the GpSimd
            # queue in block-sized pieces while f2 streams on Sync, so the
            # first activation piece's inputs land in parallel and the
            # phase-A pieces unblock one by one.
            q0_splits = [0, 256, 1024, chunks[0]]  # f2 q0 sub-transfers
            f1s = wpool.tile([128, dc, nb], mm_dtype, tag="f1s")
            f2q0 = wpool.tile(
                [128, dc, chunks[0]], mm_dtype, tag="f2q0", name="f2q0"
            )
            f2q0_done = [0] * (len(q0_splits) - 1)
            nc.gpsimd.dma_start(f1s[:, :, 0:128], f1t[:, :, 0:128])
            nc.sync.dma_start(
                f2q0[:, :, q0_splits[0] : q0_splits[1]],
                f2t[:, :, q0_splits[0] : q0_splits[1]],
            )
            for b in range(1, N_SPLIT):
                nc.gpsimd.dma_start(
                    f1s[:, :, b * 128 : (b + 1) * 128],
                    f1t[:, :, b * 128 : (b + 1) * 128],
                )
            nc.gpsimd.dma_start(
                f1s[:, :, N_SPLIT * 128 : nb], f1t[:, :, N_SPLIT * 128 : nb]
            )
            for i in range(1, len(q0_splits) - 1):
                nc.sync.dma_start(
                    f2q0[:, :, q0_splits[i] : q0_splits[i + 1]],
                    f2t[:, :, q0_splits[i] : q0_splits[i + 1]],
                )
            f2s = []
            for q in range(1, n_q):
                eng = nc.gpsimd if q % 2 else nc.sync
                f2q = wpool.tile(
                    [128, dc, chunks[q]], mm_dtype, tag=f"f2q_{q}", name=f"f2q_{q}"
                )
                eng.dma_start(f2q[:], f2t[:, :, coff[q] : coff[q + 1]])
                f2s.append(f2q)

            def rhs_slice(c0, c1, d):
                """RHS AP for matmul column group [c0, c1)."""
                if c1 <= chunks[0]:
                    return f2q0[:, d, c0:c1]
                q = c0 // qc
                r0, r1 = c0 % qc, c0 % qc + (c1 - c0)
                return f2s[q - 1][:, d, r0:r1]

            exps_t, sums_t, rsum_t, recip_t = [], [], [], []
            max_sums = n_q - 1 + len(q0_splits)
            for b in range(n_blocks):
                exps_t.append(epool.tile([128, m], f16, tag="exps", name=f"exps_{b}"))
                sums_t.append(
                    spool.tile([128, max_sums], f32, tag="sums", name=f"sums_{b}")
                )
                rsum_t.append(spool.tile([128, 1], f32, tag="rsum", name=f"rsum_{b}"))
                recip_t.append(
                    spool.tile([128, 1], f32, tag="recip", name=f"recip_{b}")
                )
            n_sums = [0] * n_blocks  # accumulator columns used per block

            def do_cols(b, c0, c1):
                """Matmul columns [c0,c1) of block b + one Exp activation.

                Column groups of <=512 (one PSUM bank each); group edges
                stay 512-aligned within the ps tile so each matmul output
                lands in a single bank."""
                w = c1 - c0
                groups = []
                g0 = 0
                while g0 < w:
                    groups.append((g0, min(g0 + 512, w)))
                    g0 += 512
                ps = ppool.tile([128, w], f32, tag="ps", name=f"ps_{b}_{c0}")
                for d in range(dc):
                    lhsT = f1s[:, d, b * 128 : (b + 1) * 128]
                    for j0, j1 in groups:
                        nc.tensor.matmul(
                            ps[:, j0:j1],
                            lhsT,
                            rhs_slice(c0 + j0, c0 + j1, d),
                            start=(d == 0),
                            stop=(d == dc - 1),
                        )
                k = n_sums[b]
                n_sums[b] += 1
                pin = ps[:, :]
                if SUMS_ON == "act":
                    nc.scalar.activation(
                        exps_t[b][:, c0:c1],
                        pin,
                        mybir.ActivationFunctionType.Exp,
                        scale=scale,
                        accum_out=sums_t[b][:, k : k + 1],
                    )
                else:
                    nc.scalar.activation(
                        exps_t[b][:, c0:c1],
                        pin,
                        mybir.ActivationFunctionType.Exp,
                        scale=scale,
                    )
                    nc.vector.reduce_sum(
                        sums_t[b][:, k : k + 1],
                        exps_t[b][:, c0:c1],
                        axis=mybir.AxisListType.X,
                    )

            out_dma_count = [0]

            def normalize_and_store(b):
                exps = exps_t[b]
                rsum, recip = rsum_t[b], recip_t[b]
                nc.vector.reduce_sum(
                    rsum[:], sums_t[b][:, 0 : n_sums[b]], axis=mybir.AxisListType.X
                )
                nc.vector.reciprocal(recip[:], rsum[:])
                for q in range(n_q):
                    sl = slice(coff[q], coff[q + 1])
                    nc.vector.tensor_scalar_mul(exps[:, sl], exps[:, sl], recip[:])
                    eng = nc.gpsimd if out_dma_count[0] % 2 else nc.sync
                    out_dma_count[0] += 1
                    eng.dma_start(out[b * 128 : (b + 1) * 128, sl], exps[:, sl])

            # Phase A: chunk q0 for every block.  The first N_SPLIT blocks
            # split q0 into pieces matching the staged f2q0 sub-transfers,
            # ordered piece-major so the ACT stream starts as soon as the
            # first 256 f2 columns land and stays gap-free while the rest
            # of the inputs stream in.
            for i in range(len(q0_splits) - 1):
                for b in range(N_SPLIT):
                    do_cols(b, q0_splits[i], q0_splits[i + 1])
            for b in range(N_SPLIT, n_blocks):
                do_cols(b, 0, chunks[0])

            # Phase B: chunks q1..q3 per block, then normalize + store as
            # soon as each block's row sums are complete, spreading the
            # output DMA over the last ~3/4 of the run.
            for b in range(n_blocks):
                for q in range(1, n_q):
                    do_cols(b, coff[q], coff[q + 1])
                normalize_and_store(b)

    nc.compile()
    return nc


_nc_cache: dict = {}


def _get_nc():
    key = (MM_DT, N_WARM, SUMS_ON, N_SPLIT)
    if key not in _nc_cache:
        _nc_cache[key] = build_nc()
    return _nc_cache[key]


def kernel(fmap1: np.ndarray, fmap2: np.ndarray) -> np.ndarray:
    f1 = np.asarray(fmap1, dtype=np.float32)
    f2 = np.asarray(fmap2, dtype=np.float32)
    np_mm = mybir.dt.np(getattr(mybir.dt, MM_DT))
    # [rows, D] -> [128, D/128, rows]: f1t[dp, dcc, n] = f1[n, dcc*128 + dp]
    f1t = np.ascontiguousarray(
        f1.T.reshape(DC, 128, N).transpose(1, 0, 2).astype(np_mm)
    )
    f2t = np.ascontiguousarray(
        f2.T.reshape(DC, 128, M).transpose(1, 0, 2).astype(np_mm)
    )

    nc = _get_nc()
    in_maps = [
        {"f1t": np.ascontiguousarray(f1t[:, :, i * NB : (i + 1) * NB]), "f2t": f2t}
        for i in range(N_CORES)
    ]
    trace = bool(os.environ.get("BASS_TRACE"))
    res = run_bass_kernel_spmd(nc, in_maps, list(range(N_CORES)), trace=trace)
    last_run_info.clear()
    last_run_info.update(
        exec_time_ns=res.exec_time_ns,
        mean_exec_time_ns=res.mean_exec_time_ns,
        profile_json=res.profile_json,
        trace_path=(res.instructions_and_trace or (None, None))[1],
    )
    return np.concatenate(
        [res.results[i]["out"] for i in range(N_CORES)], axis=0
    ).astype(np.float32)
